# revision 1
# baseline (speedup 1.0000x reference)
"""Trainium2 Bass kernel for nn_BasicBlock (MoE-combined residual conv block).

  out = relu(bn2(conv3x3(relu(bn1(conv3x3(x, w1e))), w2e)) + x)
  w{1,2}e = sum_e alpha[e] * w{1,2}[e]   (host-side: linear in weights)

Strategy (per NeuronCore, data-parallel over batch: 32 imgs -> 4 per core x 8):
  - Each conv input lives in SBUF as one zero-padded fp16 [128, 114*114]
    tile holding two row-shifted copies of the image:
      partitions 0-63  = pad(x)             ("A")
      partitions 64-127 = A shifted up 1 row (A[r+1])
    A 3x3 conv then needs 6 matmuls per output tile instead of 9:
      3 K=128 pairs (tap rows 0+1 contract together, dw=0,1,2)
      3 K=64 singles (tap row 2, read from the shifted upper half)
  - fp16 matmuls run at 1 cycle/row on the PE; accumulation is fp32 in PSUM.
  - Taps iterate outermost over a group of G=4 PSUM banks (shared 8-bank
    pool) so consecutive matmuls share stationary weights.
  - x is loaded as fp32 in row bands and cast to fp16 on DVE (banded so the
    first matmuls start early); the shifted copy is one SBUF->SBUF DMA
    issued from ACT (idle right then), so the SP sequencer never stalls
    waiting for casts.
  - conv1 lhsT duplicates the 64 output channels into M=128 so PSUM holds
    two copies; two ACT ops (bn1+relu fused, bn1 scale folded into w1)
    evict them into the A / A>>1row halves of the mid tile.
  - conv2 epilogue on DVE: residual add (fp16 x from plane A) straight out
    of PSUM, then relu (+bn2 bias; bn2 scale folded into w2), DMA to HBM.
"""

import numpy as np

import concourse.mybir as mybir
import concourse.tile as tile
from concourse import bacc
from concourse.bass_utils import run_bass_kernel_spmd

F32 = mybir.dt.float32
F16 = mybir.dt.float16
AF = mybir.ActivationFunctionType
ALU = mybir.AluOpType

EPS = 1e-5
N_CORES = 8
C = 64   # channels (in == out)
R = 4    # output rows per PSUM chunk
G = 4    # chunks per weight-stationary group
BAND = 16  # x load/cast band rows


def build_nc(B, H, W):
    """Bass program: B images of [64, H, W] per core."""
    Hp, Wp = H + 2, W + 2
    N = R * W                     # psum free size per chunk
    nchunks = H // R
    assert H % R == 0
    band = BAND if H % BAND == 0 else H
    nbands = H // band

    nc = bacc.Bacc("TRN2", target_bir_lowering=False, debug=False,
                   enable_asserts=False, num_devices=N_CORES)

    xin = nc.dram_tensor("xin", [B, C, H, W], F32, kind="ExternalInput").ap()
    w1p_d = nc.dram_tensor("w1p", [128, 3 * 128], F16, kind="ExternalInput").ap()
    w1s_d = nc.dram_tensor("w1s", [64, 3 * 128], F16, kind="ExternalInput").ap()
    w2p_d = nc.dram_tensor("w2p", [128, 3 * 64], F16, kind="ExternalInput").ap()
    w2s_d = nc.dram_tensor("w2s", [64, 3 * 64], F16, kind="ExternalInput").ap()
    b1_d = nc.dram_tensor("b1", [128, 1], F32, kind="ExternalInput").ap()
    b2_d = nc.dram_tensor("b2", [64, 1], F32, kind="ExternalInput").ap()
    yout = nc.dram_tensor("yout", [B, C, H, W], F32, kind="ExternalOutput").ap()

    with tile.TileContext(nc) as tc:
        with (
            tc.tile_pool(name="wpool", bufs=1) as wpool,
            tc.tile_pool(name="xpool", bufs=2) as xpool,
            tc.tile_pool(name="fpool", bufs=3) as fpool,
            tc.tile_pool(name="mpool", bufs=2) as mpool,
            tc.tile_pool(name="pspool", bufs=8, space="PSUM") as pspool,
            tc.tile_pool(name="upool", bufs=6) as upool,
            tc.tile_pool(name="opool", bufs=6) as opool,
        ):
            w1p = wpool.tile([128, 3 * 128], F16)
            w1s = wpool.tile([128, 3 * 128], F16)  # singles in parts 64-127
            w2p = wpool.tile([128, 3 * 64], F16)
            w2s = wpool.tile([128, 3 * 64], F16)
            b1t = wpool.tile([128, 1], F32)
            b2t = wpool.tile([64, 1], F32)
            nc.sync.dma_start(w1p[:, :], w1p_d[:, :])
            nc.sync.dma_start(w1s[64:128, :], w1s_d[:, :])
            nc.sync.dma_start(w2p[:, :], w2p_d[:, :])
            nc.sync.dma_start(w2s[64:128, :], w2s_d[:, :])
            nc.sync.dma_start(b1t[:, :], b1_d[:, :])
            nc.sync.dma_start(b2t[:, :], b2_d[:, :])

            def x_prep(img):
                """Emit input-plane construction for one image."""
                xt = xpool.tile([128, Hp * Wp], F16, tag="xt",
                                name=f"xt_{img}")
                xr = xt[:, :].rearrange("p (h w) -> p h w", w=Wp)
                nc.vector.memset(xr[0:64, 0, :], 0.0)          # A top border
                nc.vector.memset(xr[0:64, Hp - 1, :], 0.0)     # A bottom
                nc.vector.memset(xr[0:64, :, 0], 0.0)          # A left
                nc.vector.memset(xr[0:64, :, Wp - 1], 0.0)     # A right
                xin_r = xin[img]
                # image 0 is on the critical path: halve its first bands so
                # the first conv group (rows 0-19) is ready sooner
                if img == 0 and band > 8 and band % 8 == 0:
                    sizes = [8, 8, 8, 8] + [band] * (nbands - 2)
                else:
                    sizes = [band] * nbands
                r0 = 0
                for b, bsz in enumerate(sizes):
                    xf = fpool.tile([64, band * W], F32, tag="xf",
                                    name=f"xf_{img}_{b}")
                    nc.sync.dma_start(
                        xf[:, 0:bsz * W].rearrange("p (h w) -> p h w", w=W),
                        xin_r[:, r0:r0 + bsz, :])
                    nc.vector.tensor_copy(
                        xr[0:64, r0 + 1:r0 + bsz + 1, 1:W + 1],
                        xf[:, 0:bsz * W].rearrange("p (h w) -> p h w", w=W))
                    # A>>1row band copy right behind its cast (ACT-issued
                    # so SP never stalls; banded so conv1 can start after
                    # the first bands instead of the whole plane)
                    nc.scalar.dma_start(xr[64:128, r0:r0 + bsz, :],
                                        xr[0:64, r0 + 1:r0 + bsz + 1, :])
                    r0 += bsz
                # top band's row 0 of the shifted copy = A row 1..; bottom:
                nc.scalar.dma_start(xr[64:128, H:Hp - 1, :],
                                    xr[0:64, H + 1:Hp, :])
                nc.vector.memset(xr[64:128, Hp - 1, :], 0.0)
                return xr

            xr_cur = x_prep(0)
            for img in range(B):
                xr = xr_cur

                # ---- mid plane (same dual layout, written by ACT) ----
                mt = mpool.tile([128, Hp * Wp], F16, tag="mt",
                                name=f"mt_{img}")
                mr = mt[:, :].rearrange("p (h w) -> p h w", w=Wp)
                nc.vector.memset(mr[0:64, 0, :], 0.0)
                nc.vector.memset(mr[0:64, Hp - 1, :], 0.0)
                nc.vector.memset(mr[64:128, H, :], 0.0)
                nc.vector.memset(mr[:, :, 0], 0.0)
                nc.vector.memset(mr[:, :, Wp - 1], 0.0)

                # ---- conv1 + bn1 + relu -> mid (taps outer over G banks) ---
                for g0 in range(0, nchunks, G):
                    ng = min(G, nchunks - g0)
                    pss = [pspool.tile([128, N], F32, tag="ps",
                                       name=f"ps1_{img}_{g0}_{j}")
                           for j in range(ng)]
                    for dw in range(3):
                        for j in range(ng):
                            h0 = (g0 + j) * R
                            nc.tensor.matmul(
                                pss[j][:, :],
                                lhsT=w1p[:, dw * 128:(dw + 1) * 128],
                                rhs=xr[0:128, h0:h0 + R, dw:dw + W],
                                start=(dw == 0), stop=False)
                    for dw in range(3):
                        for j in range(ng):
                            h0 = (g0 + j) * R
                            nc.tensor.matmul(
                                pss[j][:, :],
                                lhsT=w1s[64:128, dw * 128:(dw + 1) * 128],
                                rhs=xr[64:128, h0 + 1:h0 + 1 + R, dw:dw + W],
                                start=False, stop=(dw == 2))
                    for j in range(ng):
                        h0 = (g0 + j) * R
                        ps1 = pss[j]
                        p1lo = ps1[0:64, :].rearrange("p (h w) -> p h w", w=W)
                        p1hi = ps1[64:128, :].rearrange("p (h w) -> p h w", w=W)
                        nc.scalar.activation(
                            mr[0:64, h0 + 1:h0 + 1 + R, 1:W + 1],
                            p1lo, AF.Relu, bias=b1t[0:64, 0:1])
                        nc.scalar.activation(
                            mr[64:128, h0:h0 + R, 1:W + 1],
                            p1hi, AF.Relu, bias=b1t[64:128, 0:1])

                # prefetch next image's input planes while conv2 runs
                if img + 1 < B:
                    xr_cur = x_prep(img + 1)

                # ---- conv2 + bn2 + residual + relu -> out ----
                for g0 in range(0, nchunks, G):
                    ng = min(G, nchunks - g0)
                    pss = [pspool.tile([64, N], F32, tag="ps",
                                       name=f"ps2_{img}_{g0}_{j}")
                           for j in range(ng)]
                    for dw in range(3):
                        for j in range(ng):
                            h0 = (g0 + j) * R
                            nc.tensor.matmul(
                                pss[j][:, :],
                                lhsT=w2p[:, dw * 64:(dw + 1) * 64],
                                rhs=mr[0:128, h0:h0 + R, dw:dw + W],
                                start=(dw == 0), stop=False)
                    for dw in range(3):
                        for j in range(ng):
                            h0 = (g0 + j) * R
                            nc.tensor.matmul(
                                pss[j][:, :],
                                lhsT=w2s[64:128, dw * 64:(dw + 1) * 64],
                                rhs=mr[64:128, h0 + 1:h0 + 1 + R, dw:dw + W],
                                start=False, stop=(dw == 2))
                    for j in range(ng):
                        h0 = (g0 + j) * R
                        ps2 = pss[j]
                        u = upool.tile([64, N], F32, tag="u",
                                       name=f"u_{img}_{g0}_{j}")
                        nc.vector.tensor_add(
                            u[:, :].rearrange("p (h w) -> p h w", w=W),
                            ps2[:, :].rearrange("p (h w) -> p h w", w=W),
                            xr[0:64, h0 + 1:h0 + 1 + R, 1:W + 1])
                        o = opool.tile([64, N], F32, tag="o",
                                       name=f"o_{img}_{g0}_{j}")
                        nc.vector.tensor_scalar(
                            o[:, :], u[:, :], b2t[:, 0:1], 0.0,
                            ALU.add, ALU.max)
                        nc.sync.dma_start(
                            yout[img][:, h0:h0 + R, :],
                            o[:, :].rearrange("p (h w) -> p h w", w=W))
    nc.compile()
    return nc


def prepare_weights(w1, w2, alpha, bn1_gamma, bn1_beta, bn1_mean, bn1_var,
                    bn2_gamma, bn2_beta, bn2_mean, bn2_var):
    w1e = np.einsum('e,eoihw->oihw', alpha.astype(np.float64),
                    w1.astype(np.float64))
    w2e = np.einsum('e,eoihw->oihw', alpha.astype(np.float64),
                    w2.astype(np.float64))
    s1 = bn1_gamma / np.sqrt(bn1_var + EPS)
    b1 = bn1_beta - bn1_mean * s1
    s2 = bn2_gamma / np.sqrt(bn2_var + EPS)
    b2 = bn2_beta - bn2_mean * s2
    w1e = (w1e * s1[:, None, None, None]).astype(np.float16)  # fold bn1 scale
    w2e = (w2e * s2[:, None, None, None]).astype(np.float16)  # fold bn2 scale

    w1p = np.zeros((128, 3 * 128), np.float16)
    w1s = np.zeros((64, 3 * 128), np.float16)
    w2p = np.zeros((128, 3 * 64), np.float16)
    w2s = np.zeros((64, 3 * 64), np.float16)
    for dw in range(3):
        for dh in (0, 1):
            w1p[dh * 64:(dh + 1) * 64, dw * 128:dw * 128 + 64] = w1e[:, :, dh, dw].T
            w1p[dh * 64:(dh + 1) * 64, dw * 128 + 64:dw * 128 + 128] = w1e[:, :, dh, dw].T
            w2p[dh * 64:(dh + 1) * 64, dw * 64:(dw + 1) * 64] = w2e[:, :, dh, dw].T
        w1s[:, dw * 128:dw * 128 + 64] = w1e[:, :, 2, dw].T
        w1s[:, dw * 128 + 64:dw * 128 + 128] = w1e[:, :, 2, dw].T
        w2s[:, dw * 64:(dw + 1) * 64] = w2e[:, :, 2, dw].T
    b1v = np.tile(b1.astype(np.float32), 2).reshape(128, 1)
    b2v = b2.astype(np.float32).reshape(64, 1)
    return w1p, w1s, w2p, w2s, b1v, b2v


_NC_CACHE = {}


def kernel(x, w1, w2, alpha,
           bn1_gamma, bn1_beta, bn1_mean, bn1_var,
           bn2_gamma, bn2_beta, bn2_mean, bn2_var):
    x = np.ascontiguousarray(np.asarray(x, dtype=np.float32))
    B_total, _, H, W = x.shape
    Bc = B_total // N_CORES
    w1p, w1s, w2p, w2s, b1v, b2v = prepare_weights(
        np.asarray(w1, np.float32), np.asarray(w2, np.float32),
        np.asarray(alpha, np.float32),
        np.asarray(bn1_gamma, np.float32), np.asarray(bn1_beta, np.float32),
        np.asarray(bn1_mean, np.float32), np.asarray(bn1_var, np.float32),
        np.asarray(bn2_gamma, np.float32), np.asarray(bn2_beta, np.float32),
        np.asarray(bn2_mean, np.float32), np.asarray(bn2_var, np.float32))

    key = (Bc, H, W)
    if key not in _NC_CACHE:
        _NC_CACHE[key] = build_nc(Bc, H, W)
    nc = _NC_CACHE[key]

    in_maps = []
    for cid in range(N_CORES):
        in_maps.append({
            "xin": x[cid * Bc:(cid + 1) * Bc],
            "w1p": w1p, "w1s": w1s, "w2p": w2p, "w2s": w2s,
            "b1": b1v, "b2": b2v,
        })
    res = run_bass_kernel_spmd(nc, in_maps, core_ids=list(range(N_CORES)))
    out = np.concatenate([res.results[cid]["yout"] for cid in range(N_CORES)],
                         axis=0)
    return out

